# revision 1
# baseline (speedup 1.0000x reference)
"""Trainium2 Bass kernel for MHA (B=2, S=2048, D=512, H=8, dk=dv=32) with additive mask.

Sharding: each of 8 cores handles one batch (c//4) and a 512-row query slice
(c%4). Host stages inputs pre-transposed + bf16 so the device needs no
transposes; scores are computed transposed ([k, q]) so the softmax sum over k
rides the PE (matmul with a ones stationary) and exp(s+m) = exp(s)*exp(m)
avoids slow PSUM-operand mask adds on DVE.
"""

import numpy as np
import ml_dtypes

B, S, D, DK, H, DH = 2, 2048, 512, 256, 8, 32
QR = S // 4  # 512 query rows per core
NCORES = 8
BF = ml_dtypes.bfloat16

_CACHED = {}


def _body(nc, tc, mybir, bass, aps):
    bf16 = mybir.dt.bfloat16
    f32 = mybir.dt.float32
    Exp = mybir.ActivationFunctionType.Exp
    qt, kt, vt, mt, wq, wk, wv, wo, bqk, bos, out = aps

    with (
        tc.tile_pool(name="cst", bufs=1) as cp,
        tc.tile_pool(name="wrk", bufs=3) as wp,
        tc.tile_pool(name="qkp", bufs=2, space="PSUM") as qkp,
        tc.tile_pool(name="avp", bufs=1, space="PSUM") as avp,
    ):
        # ---- load everything ----
        QT = cp.tile([128, 4 * QR], bf16, tag="QT")      # [Dc blocks][D_in=128, q=512]
        KT = cp.tile([128, 4 * S], bf16, tag="KT")       # [Dc][128, k=2048]
        VT = cp.tile([128, 4 * S], bf16, tag="VT")
        MT = cp.tile([128, 16 * QR], bf16, tag="MT")     # [kc][k=128, q=512]
        WQ = cp.tile([128, 4 * DK], bf16, tag="WQ")      # [Dc][128, 256]
        WK = cp.tile([128, 4 * DK], bf16, tag="WK")
        WV = cp.tile([128, 4 * DK], bf16, tag="WV")
        WO = cp.tile([128, 2 * D], bf16, tag="WO")       # [dvc][128, 512]
        BQK = cp.tile([128, 4], f32, tag="BQK")          # bq0 bq1 bk0 bk1
        BOS = cp.tile([1, D], bf16, tag="BOS")           # bo + bv@Wo
        nc.sync.dma_start(out=QT, in_=qt)
        nc.sync.dma_start(out=KT, in_=kt)
        nc.sync.dma_start(out=VT, in_=vt)
        nc.sync.dma_start(out=MT, in_=mt)
        nc.sync.dma_start(out=WQ, in_=wq)
        nc.sync.dma_start(out=WK, in_=wk)
        nc.sync.dma_start(out=WV, in_=wv)
        nc.sync.dma_start(out=WO, in_=wo)
        nc.sync.dma_start(out=BQK, in_=bqk)
        nc.sync.dma_start(out=BOS, in_=bos)

        ONES = cp.tile([128, 1], bf16, tag="ONES")
        nc.vector.memset(ONES, 1.0)
        ONE_ROW = cp.tile([128, 128], bf16, tag="ONE_ROW")
        nc.vector.memset(ONE_ROW, 1.0)

        # EM = exp(maskT), bf16, same [kc][128, 512] layout as MT
        EM = cp.tile([128, 16 * QR], bf16, tag="EM")
        nc.scalar.activation(EM, MT, Exp)

        # ---- projections ----
        # qT [dk, q]: 2 chunks of 128 dk rows; kT [dk, k]: same chunks, 2048 k
        qT = cp.tile([128, 2 * QR], bf16, tag="qT")
        kT = cp.tile([128, 2 * S], bf16, tag="kT")
        v_sb = cp.tile([128, 16 * DK], bf16, tag="v_sb")  # [kc][k=128, dv=256]
        ptags = ["avA", "avB", "avLA", "avLB"]
        pidx = 0

        for dkc in range(2):
            ps = avp.tile([128, QR], f32, tag=ptags[pidx % 4]); pidx += 1
            for Dc in range(4):
                nc.tensor.matmul(
                    ps,
                    lhsT=WQ[:, Dc * DK + dkc * 128:Dc * DK + dkc * 128 + 128],
                    rhs=QT[:, Dc * QR:(Dc + 1) * QR],
                    start=(Dc == 0), stop=(Dc == 3))
            nc.vector.tensor_scalar_add(qT[:, dkc * QR:(dkc + 1) * QR], ps,
                                        BQK[:, dkc:dkc + 1])
        for dkc in range(2):
            for kf in range(4):
                ps = avp.tile([128, 512], f32, tag=ptags[pidx % 4]); pidx += 1
                for Dc in range(4):
                    nc.tensor.matmul(
                        ps,
                        lhsT=WK[:, Dc * DK + dkc * 128:Dc * DK + dkc * 128 + 128],
                        rhs=KT[:, Dc * S + kf * 512:Dc * S + kf * 512 + 512],
                        start=(Dc == 0), stop=(Dc == 3))
                nc.vector.tensor_scalar_add(
                    kT[:, dkc * S + kf * 512:dkc * S + kf * 512 + 512], ps,
                    BQK[:, 2 + dkc:3 + dkc])
        for kc in range(16):
            ps = avp.tile([128, DK], f32, tag=ptags[pidx % 4]); pidx += 1
            for Dc in range(4):
                nc.tensor.matmul(
                    ps,
                    lhsT=VT[:, Dc * S + kc * 128:Dc * S + kc * 128 + 128],
                    rhs=WV[:, Dc * DK:(Dc + 1) * DK],
                    start=(Dc == 0), stop=(Dc == 3))
            nc.vector.tensor_copy(v_sb[:, kc * DK:(kc + 1) * DK], ps)

        # ---- attention: 4 head-pairs ----
        ctx = cp.tile([128, 2 * QR], bf16, tag="ctx")  # [dvc][dv=128, q=512]
        r_sb = cp.tile([128, QR], f32, tag="r_sb")
        rb16 = cp.tile([128, QR], bf16, tag="rb16")
        rrep = cp.tile([128, 1024], bf16, tag="rrep")

        for pair in range(4):
            hA, hB = 2 * pair, 2 * pair + 1
            cbase = 64 * (pair % 2)          # ctx col strips
            lbase = 64 - cbase               # l col strips
            avA = avp.tile([128, QR], f32, tag="avA")
            avB = avp.tile([128, QR], f32, tag="avB")
            avLA = avp.tile([128, QR], f32, tag="avLA")
            avLB = avp.tile([128, QR], f32, tag="avLB")
            for kc in range(16):
                qk = qkp.tile([128, 1024], f32, tag="qk")
                for j, h in ((0, hA), (1, hB)):
                    m = h % 4
                    dkc = h // 4
                    nc.tensor.matmul(
                        qk[:, j * 512:(j + 1) * 512],
                        lhsT=kT[32 * m:32 * m + 32,
                                dkc * S + kc * 128:dkc * S + kc * 128 + 128],
                        rhs=qT[32 * m:32 * m + 32, dkc * QR:(dkc + 1) * QR],
                        start=True, stop=True, tile_position=(32 * m, 0))
                p1 = wp.tile([128, 1024], bf16, tag="p1")
                nc.scalar.activation(p1, qk, Exp)
                p2 = wp.tile([128, 1024], bf16, tag="p2")
                em3 = EM[:, kc * QR:(kc + 1) * QR].rearrange(
                    "p (a b) -> p a b", a=1).broadcast_to((128, 2, QR))
                nc.vector.tensor_tensor(
                    out=p2.rearrange("p (a b) -> p a b", b=QR),
                    in0=p1.rearrange("p (a b) -> p a b", b=QR),
                    in1=em3, op=mybir.AluOpType.mult)
                st = (kc == 0)
                sp = (kc == 15)
                for j, h, av, avl in ((0, hA, avA, avLA), (1, hB, avB, avLB)):
                    cb = cbase + 32 * j
                    lb = lbase + 32 * j
                    nc.tensor.matmul(
                        av[cb:cb + 32, :],
                        lhsT=v_sb[:, kc * DK + DH * h:kc * DK + DH * h + DH],
                        rhs=p2[:, j * 512:(j + 1) * 512],
                        start=st, stop=sp, tile_position=(0, cb))
                    nc.tensor.matmul(
                        avl[lb:lb + 1, :],
                        lhsT=ONES,
                        rhs=p2[:, j * 512:(j + 1) * 512],
                        start=st, stop=sp, tile_position=(0, lb))
            # evacuate: r = 1/l, replicate r across 32 partitions via a K=1
            # matmul (ones stationary), then ctx_norm = ctx * r_rep
            rq = qkp.tile([128, 1024], f32, tag="qk")
            for j, h, av, avl in ((0, hA, avA, avLA), (1, hB, avB, avLB)):
                cb = cbase + 32 * j
                lb = lbase + 32 * j
                dvc = h // 4
                nc.vector.reciprocal(r_sb[lb:lb + 1, :], avl[lb:lb + 1, :])
                nc.vector.tensor_copy(rb16[lb:lb + 1, :], r_sb[lb:lb + 1, :])
                nc.tensor.matmul(
                    rq[cb:cb + 32, j * 512:(j + 1) * 512],
                    lhsT=ONE_ROW[lb:lb + 1, 0:32],
                    rhs=rb16[lb:lb + 1, :],
                    start=True, stop=True, tile_position=(lb, cb))
                nc.vector.tensor_copy(rrep[cb:cb + 32, j * 512:(j + 1) * 512],
                                      rq[cb:cb + 32, j * 512:(j + 1) * 512])
                nc.vector.tensor_mul(
                    ctx[cb:cb + 32, dvc * QR:(dvc + 1) * QR],
                    av[cb:cb + 32, :],
                    rrep[cb:cb + 32, j * 512:(j + 1) * 512])

        # ---- output projection ----
        out_sb = cp.tile([128, 4 * D], f32, tag="out_sb")
        for qc in range(4):
            ps = avp.tile([128, D], f32, tag=ptags[qc % 4])
            for dvc in range(2):
                nc.tensor.matmul(
                    ps,
                    lhsT=ctx[:, dvc * QR + qc * 128:dvc * QR + qc * 128 + 128],
                    rhs=WO[:, dvc * D:(dvc + 1) * D],
                    start=(dvc == 0), stop=False)
            nc.tensor.matmul(ps, lhsT=ONE_ROW[0:1, :], rhs=BOS,
                             start=False, stop=True)
            nc.vector.tensor_copy(out_sb[:, qc * D:(qc + 1) * D], ps)
        nc.sync.dma_start(
            out=out.rearrange("(qc p) d -> p qc d", p=128),
            in_=out_sb.rearrange("p (qc d) -> p qc d", d=D))


def _build():
    if "nc" in _CACHED:
        return _CACHED["nc"]
    import concourse.bacc as bacc
    import concourse.tile as tile
    import concourse.mybir as mybir
    import concourse.bass as bass

    bf16 = mybir.dt.bfloat16
    f32 = mybir.dt.float32
    nc = bacc.Bacc("TRN2", target_bir_lowering=False, debug=False,
                   enable_asserts=False, num_devices=NCORES)
    aps = [
        nc.dram_tensor("qt", [128, 4 * QR], bf16, kind="ExternalInput").ap(),
        nc.dram_tensor("kt", [128, 4 * S], bf16, kind="ExternalInput").ap(),
        nc.dram_tensor("vt", [128, 4 * S], bf16, kind="ExternalInput").ap(),
        nc.dram_tensor("mt", [128, 16 * QR], bf16, kind="ExternalInput").ap(),
        nc.dram_tensor("wq", [128, 4 * DK], bf16, kind="ExternalInput").ap(),
        nc.dram_tensor("wk", [128, 4 * DK], bf16, kind="ExternalInput").ap(),
        nc.dram_tensor("wv", [128, 4 * DK], bf16, kind="ExternalInput").ap(),
        nc.dram_tensor("wo", [128, 2 * D], bf16, kind="ExternalInput").ap(),
        nc.dram_tensor("bqk", [128, 4], f32, kind="ExternalInput").ap(),
        nc.dram_tensor("bos", [1, D], bf16, kind="ExternalInput").ap(),
        nc.dram_tensor("out", [QR, D], f32, kind="ExternalOutput").ap(),
    ]
    with tile.TileContext(nc) as tc:
        _body(nc, tc, mybir, bass, aps)
    nc.compile()
    _CACHED["nc"] = nc
    return nc


def _block4(x):
    # [512, C] -> [128, 4*C] with 128-row blocks side by side
    c = x.shape[1]
    return np.ascontiguousarray(
        x.reshape(4, 128, c).transpose(1, 0, 2).reshape(128, 4 * c))


def _blockn(x, nb):
    c = x.shape[1]
    return np.ascontiguousarray(
        x.reshape(nb, 128, c).transpose(1, 0, 2).reshape(128, nb * c))


def make_in_maps(V, Q, K, mask, Wq, bq, Wk, bk, Wv, bv, Wo, bo):
    f = np.float32
    V, Q, K, mask = (np.asarray(a, f) for a in (V, Q, K, mask))
    Wq, bq, Wk, bk, Wv, bv, Wo, bo = (
        np.asarray(a, f) for a in (Wq, bq, Wk, bk, Wv, bv, Wo, bo))
    denom = np.sqrt(f(DK))
    wq_h = _block4(Wq / denom).astype(BF)
    wk_h = _block4(Wk).astype(BF)
    wv_h = _block4(Wv).astype(BF)
    wo_h = _blockn(Wo, 2).astype(BF)
    bqk_h = np.ascontiguousarray(
        np.concatenate([(bq / denom).reshape(2, 128).T,
                        bk.reshape(2, 128).T], axis=1)).astype(f)
    bos_h = np.ascontiguousarray((bv @ Wo + bo).reshape(1, D)).astype(BF)

    in_maps = []
    for c in range(NCORES):
        b = c // 4
        qs = slice((c % 4) * QR, (c % 4 + 1) * QR)
        QT = np.ascontiguousarray(Q[b, qs, :].T)       # [512 D, 512 q]
        KT = np.ascontiguousarray(K[b].T)              # [512 D, 2048 k]
        VT = np.ascontiguousarray(V[b].T)
        MT = np.ascontiguousarray(mask[b, 0, qs, :].T)  # [2048 k, 512 q]
        in_maps.append({
            "qt": _block4(QT).astype(BF),
            "kt": _block4(KT).astype(BF),
            "vt": _block4(VT).astype(BF),
            "mt": _blockn(MT, 16).astype(BF),
            "wq": wq_h, "wk": wk_h, "wv": wv_h, "wo": wo_h,
            "bqk": bqk_h, "bos": bos_h,
        })
    return in_maps


def kernel(V, Q, K, mask, Wq, bq, Wk, bk, Wv, bv, Wo, bo):
    from concourse.bass_utils import run_bass_kernel_spmd
    nc = _build()
    in_maps = make_in_maps(V, Q, K, mask, Wq, bq, Wk, bk, Wv, bv, Wo, bo)
    res = run_bass_kernel_spmd(nc, in_maps, core_ids=list(range(NCORES)))
    out_full = np.empty((B, S, D), np.float32)
    for c in range(NCORES):
        out_full[c // 4, (c % 4) * QR:(c % 4 + 1) * QR, :] = \
            res.results[c]["out"]
    return out_full



# revision 20
# speedup vs baseline: 1.0607x; 1.0607x over previous
"""Trainium2 Bass kernel for MHA (B=2, S=2048, D=512, H=8, dk=dv=32) + additive mask.

Sharding: core c -> batch c//4, query slice (c%4)*512. Scores are computed
transposed ([k, q]) so softmax sums ride the PE (ones-matmul) and the AV
contraction has keys on partitions.

v2 structure (vs the v1 baseline):
- Attention runs in 2 head-group passes (heads 0-3 / 4-7) over 16 key chunks
  x 2 query halves; the 4 QK matmuls of a slot are row-tiled 4-way
  (tile_position 32j) so they stream concurrently.
- exp(scores) is split across engines: most slots use ACT (exp(s)*exp(m),
  mask exp'd on host), a configurable subset uses a fused one-op DVE
  Schraudolph approximation (bits_i16 = s*A + (m*A + B), bitcast to bf16),
  and part of the EM multiplies run on GpSimd.
- K/V projections are computed lazily inside pass A's kc loop to keep the
  PE dense (HAM stays at 2.4 GHz).
- Softmax denominators accumulate via ones-matmuls into one PSUM bank per
  pass; one reciprocal_approx_fast + one SEL-matmul replicates 1/l across
  partitions (v1 spent 32us in single-partition reciprocals here).
"""

import numpy as np
import ml_dtypes

B, S, D, DK, H, DH = 2, 2048, 512, 256, 8, 32
QR = 512
NCORES = 8
BF = ml_dtypes.bfloat16
F16h = np.float16

A16 = 184.663956  # 2^7 / ln2
C_CORR = 6.0      # Schraudolph bias correction (bf16-bit units)
B16 = 127.0 * 128.0 - C_CORR

# slot assignment: (kc, qh) -> engine for the exp/mult stage
import os as _os
_FEAT = _os.environ.get("KFEAT", "zgDOMR")
Z_SLOTS = {(kc, 1) for kc in range(1, 16, 2)} if "z" in _FEAT else set()
GPS_MULT = {(kc, 0) for kc in range(1, 16, 2)} if "g" in _FEAT else set()
F_CHUNK_DMA = "D" in _FEAT
F_F16_OUT = "O" in _FEAT
F_MEMSET = "M" in _FEAT
F_RECIP_APPROX = "R" in _FEAT

_CACHED = {}


def _body(nc, tc, mybir, aps):
    bf16 = mybir.dt.bfloat16
    f32 = mybir.dt.float32
    f16 = mybir.dt.float16
    i16 = mybir.dt.int16
    Exp = mybir.ActivationFunctionType.Exp
    Ident = mybir.ActivationFunctionType.Identity
    Alu = mybir.AluOpType
    (qt, kt, vt, em, m2, wq, wk, wv, wo, bqk, bos, sel, out) = aps

    with (
        tc.tile_pool(name="cst", bufs=1) as cp,
        tc.tile_pool(name="p1p", bufs=3) as p1p,
        tc.tile_pool(name="p2p", bufs=3) as p2p,
        tc.tile_pool(name="qkp", bufs=2, space="PSUM") as qkp,
        tc.tile_pool(name="accp", bufs=1, space="PSUM") as accp,
    ):
        # ---- persistent SBUF ----
        QT = cp.tile([128, 4 * QR], bf16, tag="QT")
        KT = cp.tile([128, 4 * S], bf16, tag="KT")       # [Dc][128, k]
        VT = cp.tile([128, 4 * S], bf16, tag="VT")
        EM = cp.tile([128, 16 * QR], bf16, tag="EM")     # [kc][128k, 512q]
        M2 = cp.tile([128, 8 * 512], f32, tag="M2")      # odd kc
        WQ = cp.tile([128, 4 * DK], bf16, tag="WQ")
        WK = cp.tile([128, 4 * DK], bf16, tag="WK")
        WV = cp.tile([128, 4 * DK], bf16, tag="WV")
        WO = cp.tile([128, 2 * D], bf16, tag="WO")
        BQK = cp.tile([128, 4], f32, tag="BQK")
        BOS = cp.tile([1, D], bf16, tag="BOS")
        SEL = cp.tile([128, 128], bf16, tag="SEL")

        nc.sync.dma_start(out=WQ, in_=wq)
        nc.sync.dma_start(out=WK, in_=wk)
        nc.sync.dma_start(out=WV, in_=wv)
        nc.sync.dma_start(out=WO, in_=wo)
        nc.sync.dma_start(out=BQK, in_=bqk)
        nc.sync.dma_start(out=BOS, in_=bos)
        nc.sync.dma_start(out=SEL, in_=sel)
        nc.sync.dma_start(out=QT, in_=qt)
        if F_CHUNK_DMA:
            # chunked loads so early kc iters start before the tail lands
            for g in range(4):
                nc.sync.dma_start(out=KT.rearrange("p (dc k) -> p dc k", dc=4)
                                  [:, :, g * 512:(g + 1) * 512],
                                  in_=kt.rearrange("p (dc k) -> p dc k", dc=4)
                                  [:, :, g * 512:(g + 1) * 512])
                nc.sync.dma_start(out=VT.rearrange("p (dc k) -> p dc k", dc=4)
                                  [:, :, g * 512:(g + 1) * 512],
                                  in_=vt.rearrange("p (dc k) -> p dc k", dc=4)
                                  [:, :, g * 512:(g + 1) * 512])
                nc.sync.dma_start(out=EM[:, g * 4 * QR:(g + 1) * 4 * QR],
                                  in_=em[:, g * 4 * QR:(g + 1) * 4 * QR])
        else:
            nc.sync.dma_start(out=KT, in_=kt)
            nc.sync.dma_start(out=VT, in_=vt)
            nc.sync.dma_start(out=EM, in_=em)
        nc.sync.dma_start(out=M2, in_=m2)

        ONES = cp.tile([128, 1], bf16, tag="ONES")
        nc.vector.memset(ONES, 1.0)
        ONE_ROW = cp.tile([1, 128], bf16, tag="ONE_ROW")
        nc.vector.memset(ONE_ROW, 1.0)

        qT = [cp.tile([128, QR], bf16, tag=f"qT{d}", name=f"qT{d}")
              for d in range(2)]
        kT = [cp.tile([128, S], bf16, tag=f"kT{d}", name=f"kT{d}")
              for d in range(2)]
        v_sb = cp.tile([128, 16 * DK], bf16, tag="v_sb")  # [kc][128k, 256dv]
        ctx = cp.tile([128, 2 * QR], bf16, tag="ctx")     # [dvc][128dv, 512q]
        r_sb = cp.tile([128, QR], f32, tag="r_sb")
        rb16 = cp.tile([128, QR], bf16, tag="rb16")
        rq_sb = cp.tile([128, QR], bf16, tag="rq_sb")
        out_sb = cp.tile([128, 4 * D], f16 if F_F16_OUT else f32,
                         tag="out_sb")

        # ---- Q projection ----
        for dkc in range(2):
            ps = qkp.tile([128, QR], f32, tag="qk")
            for Dc in range(4):
                nc.tensor.matmul(
                    ps,
                    lhsT=WQ[:, Dc * DK + dkc * 128:Dc * DK + dkc * 128 + 128],
                    rhs=QT[:, Dc * QR:(Dc + 1) * QR],
                    start=(Dc == 0), stop=(Dc == 3))
            nc.vector.tensor_scalar_add(qT[dkc], ps, BQK[:, dkc:dkc + 1])

        def project_kv(kcg):
            # kT for keys kcg*512 .. +512 (both dkc halves) + v for 4 kcs
            for dkc in range(2):
                ps = qkp.tile([128, 512], f32, tag="qk")
                for Dc in range(4):
                    nc.tensor.matmul(
                        ps,
                        lhsT=WK[:, Dc * DK + dkc * 128:Dc * DK + dkc * 128 + 128],
                        rhs=KT[:, Dc * S + kcg * 512:Dc * S + kcg * 512 + 512],
                        start=(Dc == 0), stop=(Dc == 3))
                nc.vector.tensor_scalar_add(
                    kT[dkc][:, kcg * 512:(kcg + 1) * 512], ps,
                    BQK[:, 2 + dkc:3 + dkc])
            for kk in range(4):
                kc = kcg * 4 + kk
                ps = qkp.tile([128, DK], f32, tag="qk")
                for Dc in range(4):
                    nc.tensor.matmul(
                        ps,
                        lhsT=VT[:, Dc * S + kc * 128:Dc * S + kc * 128 + 128],
                        rhs=WV[:, Dc * DK:(Dc + 1) * DK],
                        start=(Dc == 0), stop=(Dc == 3))
                nc.vector.tensor_copy(v_sb[:, kc * DK:(kc + 1) * DK], ps)

        # ---- attention: 2 passes of 4 heads ----
        # PSUM rule: every matmul output owns a full 512-col f32 bank row, so
        # each bank has exactly one start=True leader per accumulation group
        # (256-col sub-bank writes hang the device).
        for p in range(2):
            dkc = p
            av = accp.tile([128, QR], f32, tag="av")    # [4h x 32dv, 512q]
            avl = accp.tile([128, QR], f32, tag="avl")  # rows {0,32,64,96}
            # rows of avl other than {0,32,64,96} are never matmul-written
            # but are read (and discarded) by the packed reciprocal
            if F_MEMSET:
                nc.vector.memset(avl, 1.0)
            for kc in range(16):
                if p == 0 and kc % 4 == 0:
                    project_kv(kc // 4)
                for pr in range(2):
                    qk = qkp.tile([128, 1024], f32, tag="qk")
                    for jj in range(2):
                        j = 2 * pr + jj
                        nc.tensor.matmul(
                            qk[:, jj * 512:(jj + 1) * 512],
                            lhsT=kT[dkc][32 * j:32 * j + 32,
                                         kc * 128:kc * 128 + 128],
                            rhs=qT[dkc][32 * j:32 * j + 32, :],
                            start=True, stop=True, tile_position=(32 * j, 0))
                    p2 = p2p.tile([128, 1024], bf16, tag="p2")
                    if (kc, pr) in Z_SLOTS:
                        zi = (kc - 1) // 2
                        m2b = M2[:, zi * 512:(zi + 1) * 512].rearrange(
                            "p (a b) -> p a b", a=1).broadcast_to((128, 2, 512))
                        nc.vector.scalar_tensor_tensor(
                            out=p2.bitcast(i16).rearrange(
                                "p (a b) -> p a b", b=512),
                            in0=qk.rearrange("p (a b) -> p a b", b=512),
                            scalar=A16, in1=m2b, op0=Alu.mult, op1=Alu.add)
                    else:
                        p1 = p1p.tile([128, 1024], bf16, tag="p1")
                        nc.scalar.activation(p1, qk, Exp)
                        emb = EM[:, kc * QR:(kc + 1) * QR].rearrange(
                            "p (a b) -> p a b", a=1).broadcast_to((128, 2, 512))
                        eng = nc.gpsimd if (kc, pr) in GPS_MULT else nc.vector
                        eng.tensor_tensor(
                            out=p2.rearrange("p (a b) -> p a b", b=512),
                            in0=p1.rearrange("p (a b) -> p a b", b=512),
                            in1=emb, op=Alu.mult)
                    st, sp_ = (kc == 0), (kc == 15)
                    for jj in range(2):
                        j = 2 * pr + jj
                        h = 4 * p + j
                        nc.tensor.matmul(
                            av[32 * j:32 * j + 32, :],
                            lhsT=v_sb[:, kc * DK + DH * h:kc * DK + DH * h + DH],
                            rhs=p2[:, jj * 512:(jj + 1) * 512],
                            start=st, stop=sp_, tile_position=(0, 32 * j),
                            skip_group_check=True)
                    for jj in range(2):
                        j = 2 * pr + jj
                        nc.tensor.matmul(
                            avl[32 * j:32 * j + 1, :],
                            lhsT=ONES,
                            rhs=p2[:, jj * 512:(jj + 1) * 512],
                            start=st, stop=sp_, tile_position=(0, 32 * j),
                            skip_group_check=True)
            # ---- finalize pass: ctx = av / l ----
            if F_RECIP_APPROX:
                nc.vector.reciprocal_approx_fast(out=r_sb, in_=avl)
            else:
                nc.vector.reciprocal(r_sb, avl)
            nc.vector.tensor_scalar(out=rb16, in0=r_sb, scalar1=0.0,
                                    scalar2=3e38, op0=Alu.max, op1=Alu.min)
            rq = qkp.tile([128, QR], f32, tag="qk")
            nc.tensor.matmul(rq, lhsT=SEL, rhs=rb16, start=True, stop=True)
            nc.scalar.copy(rq_sb, rq)
            nc.vector.tensor_tensor(out=ctx[:, p * QR:(p + 1) * QR],
                                    in0=av, in1=rq_sb, op=Alu.mult)

        # ---- output projection ----
        for qc in range(4):
            ps = qkp.tile([128, D], f32, tag="qk")
            for dvc in range(2):
                nc.tensor.matmul(
                    ps,
                    lhsT=ctx[:, dvc * QR + qc * 128:dvc * QR + qc * 128 + 128],
                    rhs=WO[:, dvc * D:(dvc + 1) * D],
                    start=(dvc == 0), stop=False)
            nc.tensor.matmul(ps, lhsT=ONE_ROW, rhs=BOS,
                             start=False, stop=True)
            nc.scalar.copy(out_sb[:, qc * D:(qc + 1) * D], ps)
        nc.sync.dma_start(
            out=out.rearrange("(qc p) d -> p qc d", p=128),
            in_=out_sb.rearrange("p (qc d) -> p qc d", d=D))


def _build():
    if "nc" in _CACHED:
        return _CACHED["nc"]
    import concourse.bacc as bacc
    import concourse.tile as tile
    import concourse.mybir as mybir

    bf16 = mybir.dt.bfloat16
    f32 = mybir.dt.float32
    f16 = mybir.dt.float16
    nc = bacc.Bacc("TRN2", target_bir_lowering=False, debug=False,
                   enable_asserts=False, num_devices=NCORES)
    aps = [
        nc.dram_tensor("qt", [128, 4 * QR], bf16, kind="ExternalInput").ap(),
        nc.dram_tensor("kt", [128, 4 * S], bf16, kind="ExternalInput").ap(),
        nc.dram_tensor("vt", [128, 4 * S], bf16, kind="ExternalInput").ap(),
        nc.dram_tensor("em", [128, 16 * QR], bf16, kind="ExternalInput").ap(),
        nc.dram_tensor("m2", [128, 8 * 512], f32, kind="ExternalInput").ap(),
        nc.dram_tensor("wq", [128, 4 * DK], bf16, kind="ExternalInput").ap(),
        nc.dram_tensor("wk", [128, 4 * DK], bf16, kind="ExternalInput").ap(),
        nc.dram_tensor("wv", [128, 4 * DK], bf16, kind="ExternalInput").ap(),
        nc.dram_tensor("wo", [128, 2 * D], bf16, kind="ExternalInput").ap(),
        nc.dram_tensor("bqk", [128, 4], f32, kind="ExternalInput").ap(),
        nc.dram_tensor("bos", [1, D], bf16, kind="ExternalInput").ap(),
        nc.dram_tensor("sel", [128, 128], bf16, kind="ExternalInput").ap(),
        nc.dram_tensor("out", [QR, D],
                       f16 if F_F16_OUT else f32, kind="ExternalOutput").ap(),
    ]
    with tile.TileContext(nc) as tc:
        _body(nc, tc, mybir, aps)
    nc.compile()
    _CACHED["nc"] = nc
    return nc


def _block4(x):
    c = x.shape[1]
    return np.ascontiguousarray(
        x.reshape(4, 128, c).transpose(1, 0, 2).reshape(128, 4 * c))


def make_in_maps(V, Q, K, mask, Wq, bq, Wk, bk, Wv, bv, Wo, bo):
    f = np.float32
    V, Q, K, mask = (np.asarray(a, f) for a in (V, Q, K, mask))
    Wq, bq, Wk, bk, Wv, bv, Wo, bo = (
        np.asarray(a, f) for a in (Wq, bq, Wk, bk, Wv, bv, Wo, bo))
    denom = np.sqrt(f(DK))
    wq_h = _block4(Wq / denom).astype(BF)
    wk_h = _block4(Wk).astype(BF)
    wv_h = _block4(Wv).astype(BF)
    wo_h = np.ascontiguousarray(
        Wo.reshape(2, 128, D).transpose(1, 0, 2).reshape(128, 2 * D)).astype(BF)
    bqk_h = np.ascontiguousarray(
        np.concatenate([(bq / denom).reshape(2, 128).T,
                        bk.reshape(2, 128).T], axis=1)).astype(f)
    bos_h = np.ascontiguousarray((bv @ Wo + bo).reshape(1, D)).astype(BF)
    sel_h = np.zeros((128, 128), f)
    for h in range(4):
        sel_h[32 * h, 32 * h:32 * h + 32] = 1.0
    sel_h = sel_h.astype(BF)

    in_maps = []
    for c in range(NCORES):
        b = c // 4
        qs = slice((c % 4) * QR, (c % 4 + 1) * QR)
        QT = np.ascontiguousarray(Q[b, qs, :].T)        # [512 D, 512 q]
        KT = np.ascontiguousarray(K[b].T)               # [512 D, 2048 k]
        VT = np.ascontiguousarray(V[b].T)
        MT = np.ascontiguousarray(mask[b, 0, qs, :].T)  # [2048 k, 512 q]
        em_h = np.ascontiguousarray(
            np.exp(MT).reshape(16, 128, QR).transpose(1, 0, 2)
            .reshape(128, 16 * QR)).astype(BF)
        # m2 for odd kc (full q): [128, 8*512] f32
        m2_blk = MT.reshape(16, 128, QR)[1::2]  # [8, 128, 512]
        m2_h = np.ascontiguousarray(
            (m2_blk * A16 + B16).transpose(1, 0, 2).reshape(128, 8 * QR)
        ).astype(f)
        in_maps.append({
            "qt": _block4(QT).astype(BF),
            "kt": _block4(KT).astype(BF),
            "vt": _block4(VT).astype(BF),
            "em": em_h, "m2": m2_h,
            "wq": wq_h, "wk": wk_h, "wv": wv_h, "wo": wo_h,
            "bqk": bqk_h, "bos": bos_h, "sel": sel_h,
        })
    return in_maps


def kernel(V, Q, K, mask, Wq, bq, Wk, bk, Wv, bv, Wo, bo):
    from concourse.bass_utils import run_bass_kernel_spmd
    nc = _build()
    in_maps = make_in_maps(V, Q, K, mask, Wq, bq, Wk, bk, Wv, bv, Wo, bo)
    res = run_bass_kernel_spmd(nc, in_maps, core_ids=list(range(NCORES)))
    out_full = np.empty((B, S, D), np.float32)
    for c in range(NCORES):
        out_full[c // 4, (c % 4) * QR:(c % 4 + 1) * QR, :] = \
            res.results[c]["out"].astype(np.float32)
    return out_full


# revision 21
# speedup vs baseline: 1.2955x; 1.2214x over previous
"""Trainium2 Bass kernel for MHA (B=2, S=2048, D=512, H=8, dk=dv=32) + additive mask.

Sharding: core c -> batch c//4, query slice (c%4)*512. Scores are computed
transposed ([k, q]) so softmax sums ride the PE (ones-matmul) and the AV
contraction has keys on partitions.

v2 structure (vs the v1 baseline):
- Attention runs in 2 head-group passes (heads 0-3 / 4-7) over 16 key chunks
  x 2 query halves; the 4 QK matmuls of a slot are row-tiled 4-way
  (tile_position 32j) so they stream concurrently.
- exp(scores) is split across engines: most slots use ACT (exp(s)*exp(m),
  mask exp'd on host), a configurable subset uses a fused one-op DVE
  Schraudolph approximation (bits_i16 = s*A + (m*A + B), bitcast to bf16),
  and part of the EM multiplies run on GpSimd.
- K/V projections are computed lazily inside pass A's kc loop to keep the
  PE dense (HAM stays at 2.4 GHz).
- Softmax denominators accumulate via ones-matmuls into one PSUM bank per
  pass; one reciprocal_approx_fast + one SEL-matmul replicates 1/l across
  partitions (v1 spent 32us in single-partition reciprocals here).
"""

import numpy as np
import ml_dtypes

B, S, D, DK, H, DH = 2, 2048, 512, 256, 8, 32
QR = 512
NCORES = 8
BF = ml_dtypes.bfloat16
F16h = np.float16

A16 = 184.663956  # 2^7 / ln2
C_CORR = 6.0      # Schraudolph bias correction (bf16-bit units)
B16 = 127.0 * 128.0 - C_CORR

# slot assignment: (kc, qh) -> engine for the exp/mult stage
import os as _os
_FEAT = _os.environ.get("KFEAT", "zgDOMR")
Z_SLOTS = ({(kc, p, 1) for kc in range(1, 16, 2) for p in range(2)}
           if "z" in _FEAT else set())
GPS_MULT = ({(kc, p, 0) for kc in range(1, 16, 4) for p in range(2)}
            if "g" in _FEAT else set())
F_CHUNK_DMA = "D" in _FEAT
F_F16_OUT = "O" in _FEAT
F_MEMSET = "M" in _FEAT
F_RECIP_APPROX = "R" in _FEAT

_CACHED = {}


def _body(nc, tc, mybir, aps):
    bf16 = mybir.dt.bfloat16
    f32 = mybir.dt.float32
    f16 = mybir.dt.float16
    i16 = mybir.dt.int16
    Exp = mybir.ActivationFunctionType.Exp
    Ident = mybir.ActivationFunctionType.Identity
    Alu = mybir.AluOpType
    (qt, kt, vt, em, m2, wq, wk, wv, wo, bqk, bos, sel, out) = aps

    with (
        tc.tile_pool(name="cst", bufs=1) as cp,
        tc.tile_pool(name="p1p", bufs=3) as p1p,
        tc.tile_pool(name="p2p", bufs=3) as p2p,
        tc.tile_pool(name="qkp", bufs=2, space="PSUM") as qkp,
        tc.tile_pool(name="accp", bufs=1, space="PSUM") as accp,
    ):
        # ---- persistent SBUF ----
        QT = cp.tile([128, 4 * QR], bf16, tag="QT")
        KT = cp.tile([128, 4 * S], bf16, tag="KT")       # [Dc][128, k]
        VT = cp.tile([128, 4 * S], bf16, tag="VT")
        EM = cp.tile([128, 16 * QR], bf16, tag="EM")     # [kc][128k, 512q]
        M2 = cp.tile([128, 8 * 512], f32, tag="M2")      # odd kc
        WQ = cp.tile([128, 4 * DK], bf16, tag="WQ")
        WK = cp.tile([128, 4 * DK], bf16, tag="WK")
        WV = cp.tile([128, 4 * DK], bf16, tag="WV")
        WO = cp.tile([128, 2 * D], bf16, tag="WO")
        BQK = cp.tile([128, 4], f32, tag="BQK")
        BOS = cp.tile([1, D], bf16, tag="BOS")
        SEL = cp.tile([128, 128], bf16, tag="SEL")

        nc.sync.dma_start(out=WQ, in_=wq)
        nc.sync.dma_start(out=WK, in_=wk)
        nc.sync.dma_start(out=WV, in_=wv)
        nc.sync.dma_start(out=WO, in_=wo)
        nc.sync.dma_start(out=BQK, in_=bqk)
        nc.sync.dma_start(out=BOS, in_=bos)
        nc.sync.dma_start(out=SEL, in_=sel)
        nc.sync.dma_start(out=QT, in_=qt)
        if F_CHUNK_DMA:
            # chunked loads so early kc iters start before the tail lands
            for g in range(4):
                nc.sync.dma_start(out=KT.rearrange("p (dc k) -> p dc k", dc=4)
                                  [:, :, g * 512:(g + 1) * 512],
                                  in_=kt.rearrange("p (dc k) -> p dc k", dc=4)
                                  [:, :, g * 512:(g + 1) * 512])
                nc.sync.dma_start(out=VT.rearrange("p (dc k) -> p dc k", dc=4)
                                  [:, :, g * 512:(g + 1) * 512],
                                  in_=vt.rearrange("p (dc k) -> p dc k", dc=4)
                                  [:, :, g * 512:(g + 1) * 512])
                nc.sync.dma_start(out=EM[:, g * 4 * QR:(g + 1) * 4 * QR],
                                  in_=em[:, g * 4 * QR:(g + 1) * 4 * QR])
        else:
            nc.sync.dma_start(out=KT, in_=kt)
            nc.sync.dma_start(out=VT, in_=vt)
            nc.sync.dma_start(out=EM, in_=em)
        nc.sync.dma_start(out=M2, in_=m2)

        ONES = cp.tile([128, 1], bf16, tag="ONES")
        nc.vector.memset(ONES, 1.0)
        ONE_ROW = cp.tile([1, 128], bf16, tag="ONE_ROW")
        nc.vector.memset(ONE_ROW, 1.0)

        qT = [cp.tile([128, QR], bf16, tag=f"qT{d}", name=f"qT{d}")
              for d in range(2)]
        kT = [cp.tile([128, S], bf16, tag=f"kT{d}", name=f"kT{d}")
              for d in range(2)]
        v_sb = cp.tile([128, 16 * DK], bf16, tag="v_sb")  # [kc][128k, 256dv]
        ctx = cp.tile([128, 2 * QR], bf16, tag="ctx")     # [dvc][128dv, 512q]
        r_sb = cp.tile([128, QR], f32, tag="r_sb")
        rb16 = cp.tile([128, QR], bf16, tag="rb16")
        rq_sb = cp.tile([128, QR], bf16, tag="rq_sb")
        out_sb = cp.tile([128, 4 * D], f16 if F_F16_OUT else f32,
                         tag="out_sb")

        # ---- Q projection ----
        for dkc in range(2):
            ps = qkp.tile([128, QR], f32, tag="qk")
            for Dc in range(4):
                nc.tensor.matmul(
                    ps,
                    lhsT=WQ[:, Dc * DK + dkc * 128:Dc * DK + dkc * 128 + 128],
                    rhs=QT[:, Dc * QR:(Dc + 1) * QR],
                    start=(Dc == 0), stop=(Dc == 3))
            nc.vector.tensor_scalar_add(qT[dkc], ps, BQK[:, dkc:dkc + 1])

        def project_kv(kcg):
            # kT for keys kcg*512 .. +512 (both dkc halves) + v for 4 kcs
            for dkc in range(2):
                ps = qkp.tile([128, 512], f32, tag="qk")
                for Dc in range(4):
                    nc.tensor.matmul(
                        ps,
                        lhsT=WK[:, Dc * DK + dkc * 128:Dc * DK + dkc * 128 + 128],
                        rhs=KT[:, Dc * S + kcg * 512:Dc * S + kcg * 512 + 512],
                        start=(Dc == 0), stop=(Dc == 3))
                nc.vector.tensor_scalar_add(
                    kT[dkc][:, kcg * 512:(kcg + 1) * 512], ps,
                    BQK[:, 2 + dkc:3 + dkc])
            for kk in range(4):
                kc = kcg * 4 + kk
                ps = qkp.tile([128, DK], f32, tag="qk")
                for Dc in range(4):
                    nc.tensor.matmul(
                        ps,
                        lhsT=VT[:, Dc * S + kc * 128:Dc * S + kc * 128 + 128],
                        rhs=WV[:, Dc * DK:(Dc + 1) * DK],
                        start=(Dc == 0), stop=(Dc == 3))
                nc.vector.tensor_copy(v_sb[:, kc * DK:(kc + 1) * DK], ps)

        # ---- attention: both head-group passes interleaved per kc ----
        # PSUM rule: every matmul output owns a full 512-col f32 bank row
        # (256-col sub-bank writes hang the device). Interleaving the two
        # passes keeps the PE densely busy so the HAM clock stays at 2.4GHz.
        avs, avls = {}, {}
        for p in range(2):
            avs[p] = accp.tile([128, QR], f32, tag=f"av{p}", name=f"av{p}")
            avls[p] = accp.tile([128, QR], f32, tag=f"avl{p}", name=f"avl{p}")
            if F_MEMSET:
                # rows other than {0,32,64,96} are never matmul-written but
                # are read (and discarded) by the packed reciprocal
                nc.vector.memset(avls[p], 1.0)
        for kc in range(16):
            if kc % 4 == 0:
                project_kv(kc // 4)
            st, sp_ = (kc == 0), (kc == 15)
            for p in range(2):
                dkc = p
                av, avl = avs[p], avls[p]
                p2s = []
                for pr in range(2):
                    qk = qkp.tile([128, 1024], f32, tag="qk")
                    for jj in range(2):
                        j = 2 * pr + jj
                        nc.tensor.matmul(
                            qk[:, jj * 512:(jj + 1) * 512],
                            lhsT=kT[dkc][32 * j:32 * j + 32,
                                         kc * 128:kc * 128 + 128],
                            rhs=qT[dkc][32 * j:32 * j + 32, :],
                            start=True, stop=True, tile_position=(32 * j, 0))
                    p2 = p2p.tile([128, 1024], bf16, tag="p2")
                    if (kc, p, pr) in Z_SLOTS:
                        zi = (kc - 1) // 2
                        m2b = M2[:, zi * 512:(zi + 1) * 512].rearrange(
                            "p (a b) -> p a b", a=1).broadcast_to((128, 2, 512))
                        nc.vector.scalar_tensor_tensor(
                            out=p2.bitcast(i16).rearrange(
                                "p (a b) -> p a b", b=512),
                            in0=qk.rearrange("p (a b) -> p a b", b=512),
                            scalar=A16, in1=m2b, op0=Alu.mult, op1=Alu.add)
                    else:
                        p1 = p1p.tile([128, 1024], bf16, tag="p1")
                        nc.scalar.activation(p1, qk, Exp)
                        emb = EM[:, kc * QR:(kc + 1) * QR].rearrange(
                            "p (a b) -> p a b", a=1).broadcast_to((128, 2, 512))
                        eng = (nc.gpsimd if (kc, p, pr) in GPS_MULT
                               else nc.vector)
                        eng.tensor_tensor(
                            out=p2.rearrange("p (a b) -> p a b", b=512),
                            in0=p1.rearrange("p (a b) -> p a b", b=512),
                            in1=emb, op=Alu.mult)
                    p2s.append(p2)
                for j in range(4):
                    h = 4 * p + j
                    nc.tensor.matmul(
                        av[32 * j:32 * j + 32, :],
                        lhsT=v_sb[:, kc * DK + DH * h:kc * DK + DH * h + DH],
                        rhs=p2s[j // 2][:, (j % 2) * 512:(j % 2) * 512 + 512],
                        start=st, stop=sp_, tile_position=(0, 32 * j),
                        skip_group_check=True)
                for j in range(4):
                    nc.tensor.matmul(
                        avl[32 * j:32 * j + 1, :],
                        lhsT=ONES,
                        rhs=p2s[j // 2][:, (j % 2) * 512:(j % 2) * 512 + 512],
                        start=st, stop=sp_, tile_position=(0, 32 * j),
                        skip_group_check=True)
        # ---- finalize: ctx = av / l ----
        for p in range(2):
            av, avl = avs[p], avls[p]
            if F_RECIP_APPROX:
                nc.vector.reciprocal_approx_fast(out=r_sb, in_=avl)
            else:
                nc.vector.reciprocal(r_sb, avl)
            nc.vector.tensor_scalar(out=rb16, in0=r_sb, scalar1=0.0,
                                    scalar2=3e38, op0=Alu.max, op1=Alu.min)
            rq = qkp.tile([128, QR], f32, tag="qk")
            nc.tensor.matmul(rq, lhsT=SEL, rhs=rb16, start=True, stop=True)
            nc.scalar.copy(rq_sb, rq)
            nc.vector.tensor_tensor(out=ctx[:, p * QR:(p + 1) * QR],
                                    in0=av, in1=rq_sb, op=Alu.mult)

        # ---- output projection ----
        for qc in range(4):
            ps = qkp.tile([128, D], f32, tag="qk")
            for dvc in range(2):
                nc.tensor.matmul(
                    ps,
                    lhsT=ctx[:, dvc * QR + qc * 128:dvc * QR + qc * 128 + 128],
                    rhs=WO[:, dvc * D:(dvc + 1) * D],
                    start=(dvc == 0), stop=False)
            nc.tensor.matmul(ps, lhsT=ONE_ROW, rhs=BOS,
                             start=False, stop=True)
            nc.scalar.copy(out_sb[:, qc * D:(qc + 1) * D], ps)
        nc.sync.dma_start(
            out=out.rearrange("(qc p) d -> p qc d", p=128),
            in_=out_sb.rearrange("p (qc d) -> p qc d", d=D))


def _build():
    if "nc" in _CACHED:
        return _CACHED["nc"]
    import concourse.bacc as bacc
    import concourse.tile as tile
    import concourse.mybir as mybir

    bf16 = mybir.dt.bfloat16
    f32 = mybir.dt.float32
    f16 = mybir.dt.float16
    nc = bacc.Bacc("TRN2", target_bir_lowering=False, debug=False,
                   enable_asserts=False, num_devices=NCORES)
    aps = [
        nc.dram_tensor("qt", [128, 4 * QR], bf16, kind="ExternalInput").ap(),
        nc.dram_tensor("kt", [128, 4 * S], bf16, kind="ExternalInput").ap(),
        nc.dram_tensor("vt", [128, 4 * S], bf16, kind="ExternalInput").ap(),
        nc.dram_tensor("em", [128, 16 * QR], bf16, kind="ExternalInput").ap(),
        nc.dram_tensor("m2", [128, 8 * 512], f32, kind="ExternalInput").ap(),
        nc.dram_tensor("wq", [128, 4 * DK], bf16, kind="ExternalInput").ap(),
        nc.dram_tensor("wk", [128, 4 * DK], bf16, kind="ExternalInput").ap(),
        nc.dram_tensor("wv", [128, 4 * DK], bf16, kind="ExternalInput").ap(),
        nc.dram_tensor("wo", [128, 2 * D], bf16, kind="ExternalInput").ap(),
        nc.dram_tensor("bqk", [128, 4], f32, kind="ExternalInput").ap(),
        nc.dram_tensor("bos", [1, D], bf16, kind="ExternalInput").ap(),
        nc.dram_tensor("sel", [128, 128], bf16, kind="ExternalInput").ap(),
        nc.dram_tensor("out", [QR, D],
                       f16 if F_F16_OUT else f32, kind="ExternalOutput").ap(),
    ]
    with tile.TileContext(nc) as tc:
        _body(nc, tc, mybir, aps)
    nc.compile()
    _CACHED["nc"] = nc
    return nc


def _block4(x):
    c = x.shape[1]
    return np.ascontiguousarray(
        x.reshape(4, 128, c).transpose(1, 0, 2).reshape(128, 4 * c))


def make_in_maps(V, Q, K, mask, Wq, bq, Wk, bk, Wv, bv, Wo, bo):
    f = np.float32
    V, Q, K, mask = (np.asarray(a, f) for a in (V, Q, K, mask))
    Wq, bq, Wk, bk, Wv, bv, Wo, bo = (
        np.asarray(a, f) for a in (Wq, bq, Wk, bk, Wv, bv, Wo, bo))
    denom = np.sqrt(f(DK))
    wq_h = _block4(Wq / denom).astype(BF)
    wk_h = _block4(Wk).astype(BF)
    wv_h = _block4(Wv).astype(BF)
    wo_h = np.ascontiguousarray(
        Wo.reshape(2, 128, D).transpose(1, 0, 2).reshape(128, 2 * D)).astype(BF)
    bqk_h = np.ascontiguousarray(
        np.concatenate([(bq / denom).reshape(2, 128).T,
                        bk.reshape(2, 128).T], axis=1)).astype(f)
    bos_h = np.ascontiguousarray((bv @ Wo + bo).reshape(1, D)).astype(BF)
    sel_h = np.zeros((128, 128), f)
    for h in range(4):
        sel_h[32 * h, 32 * h:32 * h + 32] = 1.0
    sel_h = sel_h.astype(BF)

    in_maps = []
    for c in range(NCORES):
        b = c // 4
        qs = slice((c % 4) * QR, (c % 4 + 1) * QR)
        QT = np.ascontiguousarray(Q[b, qs, :].T)        # [512 D, 512 q]
        KT = np.ascontiguousarray(K[b].T)               # [512 D, 2048 k]
        VT = np.ascontiguousarray(V[b].T)
        MT = np.ascontiguousarray(mask[b, 0, qs, :].T)  # [2048 k, 512 q]
        em_h = np.ascontiguousarray(
            np.exp(MT).reshape(16, 128, QR).transpose(1, 0, 2)
            .reshape(128, 16 * QR)).astype(BF)
        # m2 for odd kc (full q): [128, 8*512] f32
        m2_blk = MT.reshape(16, 128, QR)[1::2]  # [8, 128, 512]
        m2_h = np.ascontiguousarray(
            (m2_blk * A16 + B16).transpose(1, 0, 2).reshape(128, 8 * QR)
        ).astype(f)
        in_maps.append({
            "qt": _block4(QT).astype(BF),
            "kt": _block4(KT).astype(BF),
            "vt": _block4(VT).astype(BF),
            "em": em_h, "m2": m2_h,
            "wq": wq_h, "wk": wk_h, "wv": wv_h, "wo": wo_h,
            "bqk": bqk_h, "bos": bos_h, "sel": sel_h,
        })
    return in_maps


def kernel(V, Q, K, mask, Wq, bq, Wk, bk, Wv, bv, Wo, bo):
    from concourse.bass_utils import run_bass_kernel_spmd
    nc = _build()
    in_maps = make_in_maps(V, Q, K, mask, Wq, bq, Wk, bk, Wv, bv, Wo, bo)
    res = run_bass_kernel_spmd(nc, in_maps, core_ids=list(range(NCORES)))
    out_full = np.empty((B, S, D), np.float32)
    for c in range(NCORES):
        out_full[c // 4, (c % 4) * QR:(c % 4 + 1) * QR, :] = \
            res.results[c]["out"].astype(np.float32)
    return out_full


# revision 22
# speedup vs baseline: 1.4139x; 1.0913x over previous
"""Trainium2 Bass kernel for MHA (B=2, S=2048, D=512, H=8, dk=dv=32) + additive mask.

Sharding: core c -> batch c//4, query slice (c%4)*512. Scores are computed
transposed ([k, q]) so softmax sums ride the PE (ones-matmul) and the AV
contraction has keys on partitions.

v2 structure (vs the v1 baseline):
- Attention runs in 2 head-group passes (heads 0-3 / 4-7) over 16 key chunks
  x 2 query halves; the 4 QK matmuls of a slot are row-tiled 4-way
  (tile_position 32j) so they stream concurrently.
- exp(scores) is split across engines: most slots use ACT (exp(s)*exp(m),
  mask exp'd on host), a configurable subset uses a fused one-op DVE
  Schraudolph approximation (bits_i16 = s*A + (m*A + B), bitcast to bf16),
  and part of the EM multiplies run on GpSimd.
- K/V projections are computed lazily inside pass A's kc loop to keep the
  PE dense (HAM stays at 2.4 GHz).
- Softmax denominators accumulate via ones-matmuls into one PSUM bank per
  pass; one reciprocal_approx_fast + one SEL-matmul replicates 1/l across
  partitions (v1 spent 32us in single-partition reciprocals here).
"""

import numpy as np
import ml_dtypes

B, S, D, DK, H, DH = 2, 2048, 512, 256, 8, 32
QR = 512
NCORES = 8
BF = ml_dtypes.bfloat16
F16h = np.float16

A16 = 184.663956  # 2^7 / ln2
C_CORR = 6.0      # Schraudolph bias correction (bf16-bit units)
B16 = 127.0 * 128.0 - C_CORR

# slot assignment: (kc, qh) -> engine for the exp/mult stage
import os as _os
_FEAT = _os.environ.get("KFEAT", "zgDOMR")
Z_SLOTS = ({(kc, p, pr) for kc in range(9, 16, 2) for p in range(2)
            for pr in range(2)} if "z" in _FEAT else set())
GPS_MULT = ({(kc, p, 0) for kc in range(2, 16, 4) for p in range(2)}
            if "g" in _FEAT else set())
F_CHUNK_DMA = "D" in _FEAT
F_F16_OUT = "O" in _FEAT
F_MEMSET = "M" in _FEAT
F_RECIP_APPROX = "R" in _FEAT

_CACHED = {}


def _body(nc, tc, mybir, aps):
    bf16 = mybir.dt.bfloat16
    f32 = mybir.dt.float32
    f16 = mybir.dt.float16
    i16 = mybir.dt.int16
    Exp = mybir.ActivationFunctionType.Exp
    Ident = mybir.ActivationFunctionType.Identity
    Alu = mybir.AluOpType
    (qt, kt, vt, em, m2, wq, wk, wv, wo, bqk, bos, sel, out) = aps

    with (
        tc.tile_pool(name="cst", bufs=1) as cp,
        tc.tile_pool(name="p1p", bufs=3) as p1p,
        tc.tile_pool(name="p2p", bufs=3) as p2p,
        tc.tile_pool(name="qkp", bufs=2, space="PSUM") as qkp,
        tc.tile_pool(name="accp", bufs=1, space="PSUM") as accp,
    ):
        # ---- persistent SBUF ----
        QT = cp.tile([128, 4 * QR], bf16, tag="QT")
        KT = cp.tile([128, 4 * S], bf16, tag="KT")       # [Dc][128, k]
        VT = cp.tile([128, 4 * S], bf16, tag="VT")
        EM = cp.tile([128, 16 * QR], bf16, tag="EM")     # [kc][128k, 512q]
        M2 = cp.tile([128, 4 * 512], f32, tag="M2")      # kc 9,11,13,15
        WQ = cp.tile([128, 4 * DK], bf16, tag="WQ")
        WK = cp.tile([128, 4 * DK], bf16, tag="WK")
        WV = cp.tile([128, 4 * DK], bf16, tag="WV")
        WO = cp.tile([128, 2 * D], bf16, tag="WO")
        BQK = cp.tile([128, 4], f32, tag="BQK")
        BOS = cp.tile([1, D], bf16, tag="BOS")
        SEL = cp.tile([128, 128], bf16, tag="SEL")

        # loads ordered by first use; big tensors chunked so early kc
        # iterations start before the tail lands
        nc.sync.dma_start(out=WQ, in_=wq)
        nc.sync.dma_start(out=QT, in_=qt)
        nc.sync.dma_start(out=WK, in_=wk)
        nc.sync.dma_start(out=WV, in_=wv)
        nc.sync.dma_start(out=BQK, in_=bqk)
        if F_CHUNK_DMA:
            for g in range(4):
                nc.sync.dma_start(out=KT.rearrange("p (dc k) -> p dc k", dc=4)
                                  [:, :, g * 512:(g + 1) * 512],
                                  in_=kt.rearrange("p (dc k) -> p dc k", dc=4)
                                  [:, :, g * 512:(g + 1) * 512])
                nc.sync.dma_start(out=VT.rearrange("p (dc k) -> p dc k", dc=4)
                                  [:, :, g * 512:(g + 1) * 512],
                                  in_=vt.rearrange("p (dc k) -> p dc k", dc=4)
                                  [:, :, g * 512:(g + 1) * 512])
                nc.sync.dma_start(out=EM[:, g * 4 * QR:(g + 1) * 4 * QR],
                                  in_=em[:, g * 4 * QR:(g + 1) * 4 * QR])
        else:
            nc.sync.dma_start(out=KT, in_=kt)
            nc.sync.dma_start(out=VT, in_=vt)
            nc.sync.dma_start(out=EM, in_=em)
        nc.sync.dma_start(out=SEL, in_=sel)
        nc.sync.dma_start(out=WO, in_=wo)
        nc.sync.dma_start(out=BOS, in_=bos)
        for g in range(2):
            nc.sync.dma_start(out=M2[:, g * 1024:(g + 1) * 1024],
                              in_=m2[:, g * 1024:(g + 1) * 1024])
        ONES = cp.tile([128, 1], bf16, tag="ONES")
        nc.vector.memset(ONES, 1.0)
        ONE_ROW = cp.tile([1, 128], bf16, tag="ONE_ROW")
        nc.vector.memset(ONE_ROW, 1.0)

        qT = [cp.tile([128, QR], bf16, tag=f"qT{d}", name=f"qT{d}")
              for d in range(2)]
        kT = [cp.tile([128, S], bf16, tag=f"kT{d}", name=f"kT{d}")
              for d in range(2)]
        v_sb = cp.tile([128, 16 * DK], bf16, tag="v_sb")  # [kc][128k, 256dv]
        ctx = cp.tile([128, 2 * QR], bf16, tag="ctx")     # [dvc][128dv, 512q]
        r_sb = cp.tile([128, QR], f32, tag="r_sb")
        rb16 = cp.tile([128, QR], bf16, tag="rb16")
        rq_sb = cp.tile([128, QR], bf16, tag="rq_sb")
        out_sb = cp.tile([128, 4 * D], f16 if F_F16_OUT else f32,
                         tag="out_sb")

        # ---- Q projection ----
        for dkc in range(2):
            ps = qkp.tile([128, QR], f32, tag="qk")
            for Dc in range(4):
                nc.tensor.matmul(
                    ps,
                    lhsT=WQ[:, Dc * DK + dkc * 128:Dc * DK + dkc * 128 + 128],
                    rhs=QT[:, Dc * QR:(Dc + 1) * QR],
                    start=(Dc == 0), stop=(Dc == 3))
            nc.vector.tensor_scalar_add(qT[dkc], ps, BQK[:, dkc:dkc + 1])

        def project_kv(kcg):
            # kT for keys kcg*512 .. +512 (both dkc halves) + v for 4 kcs
            for dkc in range(2):
                ps = qkp.tile([128, 512], f32, tag="qk")
                for Dc in range(4):
                    nc.tensor.matmul(
                        ps,
                        lhsT=WK[:, Dc * DK + dkc * 128:Dc * DK + dkc * 128 + 128],
                        rhs=KT[:, Dc * S + kcg * 512:Dc * S + kcg * 512 + 512],
                        start=(Dc == 0), stop=(Dc == 3))
                nc.vector.tensor_scalar_add(
                    kT[dkc][:, kcg * 512:(kcg + 1) * 512], ps,
                    BQK[:, 2 + dkc:3 + dkc])
            for kk in range(4):
                kc = kcg * 4 + kk
                ps = qkp.tile([128, DK], f32, tag="qk")
                for Dc in range(4):
                    nc.tensor.matmul(
                        ps,
                        lhsT=VT[:, Dc * S + kc * 128:Dc * S + kc * 128 + 128],
                        rhs=WV[:, Dc * DK:(Dc + 1) * DK],
                        start=(Dc == 0), stop=(Dc == 3))
                nc.vector.tensor_copy(v_sb[:, kc * DK:(kc + 1) * DK], ps)

        # ---- attention: both head-group passes interleaved per kc ----
        # PSUM rule: every matmul output owns a full 512-col f32 bank row
        # (256-col sub-bank writes hang the device). Interleaving the two
        # passes keeps the PE densely busy so the HAM clock stays at 2.4GHz.
        avs, avls = {}, {}
        for p in range(2):
            avs[p] = accp.tile([128, QR], f32, tag=f"av{p}", name=f"av{p}")
            avls[p] = accp.tile([128, QR], f32, tag=f"avl{p}", name=f"avl{p}")
            if F_MEMSET:
                # rows other than {0,32,64,96} are never matmul-written but
                # are read (and discarded) by the packed reciprocal
                nc.vector.memset(avls[p], 1.0)
        for kc in range(16):
            if kc % 4 == 0:
                project_kv(kc // 4)
            st, sp_ = (kc == 0), (kc == 15)
            for p in range(2):
                dkc = p
                av, avl = avs[p], avls[p]
                p2s = []
                for pr in range(2):
                    qk = qkp.tile([128, 1024], f32, tag="qk")
                    for jj in range(2):
                        j = 2 * pr + jj
                        nc.tensor.matmul(
                            qk[:, jj * 512:(jj + 1) * 512],
                            lhsT=kT[dkc][32 * j:32 * j + 32,
                                         kc * 128:kc * 128 + 128],
                            rhs=qT[dkc][32 * j:32 * j + 32, :],
                            start=True, stop=True, tile_position=(32 * j, 0))
                    p2 = p2p.tile([128, 1024], bf16, tag="p2")
                    if (kc, p, pr) in Z_SLOTS:
                        zi = (kc - 9) // 2
                        m2b = M2[:, zi * 512:(zi + 1) * 512].rearrange(
                            "p (a b) -> p a b", a=1).broadcast_to((128, 2, 512))
                        nc.vector.scalar_tensor_tensor(
                            out=p2.bitcast(i16).rearrange(
                                "p (a b) -> p a b", b=512),
                            in0=qk.rearrange("p (a b) -> p a b", b=512),
                            scalar=A16, in1=m2b, op0=Alu.mult, op1=Alu.add)
                    else:
                        p1 = p1p.tile([128, 1024], bf16, tag="p1")
                        nc.scalar.activation(p1, qk, Exp)
                        emb = EM[:, kc * QR:(kc + 1) * QR].rearrange(
                            "p (a b) -> p a b", a=1).broadcast_to((128, 2, 512))
                        eng = (nc.gpsimd if (kc, p, pr) in GPS_MULT
                               else nc.vector)
                        eng.tensor_tensor(
                            out=p2.rearrange("p (a b) -> p a b", b=512),
                            in0=p1.rearrange("p (a b) -> p a b", b=512),
                            in1=emb, op=Alu.mult)
                    p2s.append(p2)
                for j in range(4):
                    h = 4 * p + j
                    nc.tensor.matmul(
                        av[32 * j:32 * j + 32, :],
                        lhsT=v_sb[:, kc * DK + DH * h:kc * DK + DH * h + DH],
                        rhs=p2s[j // 2][:, (j % 2) * 512:(j % 2) * 512 + 512],
                        start=st, stop=sp_, tile_position=(0, 32 * j),
                        skip_group_check=True)
                for j in range(4):
                    nc.tensor.matmul(
                        avl[32 * j:32 * j + 1, :],
                        lhsT=ONES,
                        rhs=p2s[j // 2][:, (j % 2) * 512:(j % 2) * 512 + 512],
                        start=st, stop=sp_, tile_position=(0, 32 * j),
                        skip_group_check=True)
        # ---- finalize: ctx = av / l ----
        for p in range(2):
            av, avl = avs[p], avls[p]
            if F_RECIP_APPROX:
                nc.vector.reciprocal_approx_fast(out=r_sb, in_=avl)
            else:
                nc.vector.reciprocal(r_sb, avl)
            nc.vector.tensor_scalar(out=rb16, in0=r_sb, scalar1=0.0,
                                    scalar2=3e38, op0=Alu.max, op1=Alu.min)
            rq = qkp.tile([128, QR], f32, tag="qk")
            nc.tensor.matmul(rq, lhsT=SEL, rhs=rb16, start=True, stop=True)
            nc.scalar.copy(rq_sb, rq)
            nc.vector.tensor_tensor(out=ctx[:, p * QR:(p + 1) * QR],
                                    in0=av, in1=rq_sb, op=Alu.mult)

        # ---- output projection ----
        for qc in range(4):
            ps = qkp.tile([128, D], f32, tag="qk")
            for dvc in range(2):
                nc.tensor.matmul(
                    ps,
                    lhsT=ctx[:, dvc * QR + qc * 128:dvc * QR + qc * 128 + 128],
                    rhs=WO[:, dvc * D:(dvc + 1) * D],
                    start=(dvc == 0), stop=False)
            nc.tensor.matmul(ps, lhsT=ONE_ROW, rhs=BOS,
                             start=False, stop=True)
            nc.scalar.copy(out_sb[:, qc * D:(qc + 1) * D], ps)
        nc.sync.dma_start(
            out=out.rearrange("(qc p) d -> p qc d", p=128),
            in_=out_sb.rearrange("p (qc d) -> p qc d", d=D))


def _build():
    if "nc" in _CACHED:
        return _CACHED["nc"]
    import concourse.bacc as bacc
    import concourse.tile as tile
    import concourse.mybir as mybir

    bf16 = mybir.dt.bfloat16
    f32 = mybir.dt.float32
    f16 = mybir.dt.float16
    nc = bacc.Bacc("TRN2", target_bir_lowering=False, debug=False,
                   enable_asserts=False, num_devices=NCORES)
    aps = [
        nc.dram_tensor("qt", [128, 4 * QR], bf16, kind="ExternalInput").ap(),
        nc.dram_tensor("kt", [128, 4 * S], bf16, kind="ExternalInput").ap(),
        nc.dram_tensor("vt", [128, 4 * S], bf16, kind="ExternalInput").ap(),
        nc.dram_tensor("em", [128, 16 * QR], bf16, kind="ExternalInput").ap(),
        nc.dram_tensor("m2", [128, 4 * 512], f32, kind="ExternalInput").ap(),
        nc.dram_tensor("wq", [128, 4 * DK], bf16, kind="ExternalInput").ap(),
        nc.dram_tensor("wk", [128, 4 * DK], bf16, kind="ExternalInput").ap(),
        nc.dram_tensor("wv", [128, 4 * DK], bf16, kind="ExternalInput").ap(),
        nc.dram_tensor("wo", [128, 2 * D], bf16, kind="ExternalInput").ap(),
        nc.dram_tensor("bqk", [128, 4], f32, kind="ExternalInput").ap(),
        nc.dram_tensor("bos", [1, D], bf16, kind="ExternalInput").ap(),
        nc.dram_tensor("sel", [128, 128], bf16, kind="ExternalInput").ap(),
        nc.dram_tensor("out", [QR, D],
                       f16 if F_F16_OUT else f32, kind="ExternalOutput").ap(),
    ]
    with tile.TileContext(nc) as tc:
        _body(nc, tc, mybir, aps)
    nc.compile()
    _CACHED["nc"] = nc
    return nc


def _block4(x):
    c = x.shape[1]
    return np.ascontiguousarray(
        x.reshape(4, 128, c).transpose(1, 0, 2).reshape(128, 4 * c))


def make_in_maps(V, Q, K, mask, Wq, bq, Wk, bk, Wv, bv, Wo, bo):
    f = np.float32
    V, Q, K, mask = (np.asarray(a, f) for a in (V, Q, K, mask))
    Wq, bq, Wk, bk, Wv, bv, Wo, bo = (
        np.asarray(a, f) for a in (Wq, bq, Wk, bk, Wv, bv, Wo, bo))
    denom = np.sqrt(f(DK))
    wq_h = _block4(Wq / denom).astype(BF)
    wk_h = _block4(Wk).astype(BF)
    wv_h = _block4(Wv).astype(BF)
    wo_h = np.ascontiguousarray(
        Wo.reshape(2, 128, D).transpose(1, 0, 2).reshape(128, 2 * D)).astype(BF)
    bqk_h = np.ascontiguousarray(
        np.concatenate([(bq / denom).reshape(2, 128).T,
                        bk.reshape(2, 128).T], axis=1)).astype(f)
    bos_h = np.ascontiguousarray((bv @ Wo + bo).reshape(1, D)).astype(BF)
    sel_h = np.zeros((128, 128), f)
    for h in range(4):
        sel_h[32 * h, 32 * h:32 * h + 32] = 1.0
    sel_h = sel_h.astype(BF)

    in_maps = []
    for c in range(NCORES):
        b = c // 4
        qs = slice((c % 4) * QR, (c % 4 + 1) * QR)
        QT = np.ascontiguousarray(Q[b, qs, :].T)        # [512 D, 512 q]
        KT = np.ascontiguousarray(K[b].T)               # [512 D, 2048 k]
        VT = np.ascontiguousarray(V[b].T)
        MT = np.ascontiguousarray(mask[b, 0, qs, :].T)  # [2048 k, 512 q]
        em_h = np.ascontiguousarray(
            np.exp(MT).reshape(16, 128, QR).transpose(1, 0, 2)
            .reshape(128, 16 * QR)).astype(BF)
        # m2 for kc in {9,11,13,15} (full q): [128, 4*512] f32
        m2_blk = MT.reshape(16, 128, QR)[9::2]  # [4, 128, 512]
        m2_h = np.ascontiguousarray(
            (m2_blk * A16 + B16).transpose(1, 0, 2).reshape(128, 4 * QR)
        ).astype(f)
        in_maps.append({
            "qt": _block4(QT).astype(BF),
            "kt": _block4(KT).astype(BF),
            "vt": _block4(VT).astype(BF),
            "em": em_h, "m2": m2_h,
            "wq": wq_h, "wk": wk_h, "wv": wv_h, "wo": wo_h,
            "bqk": bqk_h, "bos": bos_h, "sel": sel_h,
        })
    return in_maps


def kernel(V, Q, K, mask, Wq, bq, Wk, bk, Wv, bv, Wo, bo):
    from concourse.bass_utils import run_bass_kernel_spmd
    nc = _build()
    in_maps = make_in_maps(V, Q, K, mask, Wq, bq, Wk, bk, Wv, bv, Wo, bo)
    res = run_bass_kernel_spmd(nc, in_maps, core_ids=list(range(NCORES)))
    out_full = np.empty((B, S, D), np.float32)
    for c in range(NCORES):
        out_full[c // 4, (c % 4) * QR:(c % 4 + 1) * QR, :] = \
            res.results[c]["out"].astype(np.float32)
    return out_full
